# revision 10
# baseline (speedup 1.0000x reference)
"""Causal self-attention (B=2, T=2048, C=1024, H=16) on 8 Trainium2 cores.

Sharding: tensor-parallel over heads (2 heads/core). Each core computes
QKV projection for its heads, causal attention, and a partial c_proj
output; partials are summed on the host (b_proj is added by core 0 only).

Per-core dataflow (everything kept "K-major" so no activation transposes
are needed on the critical path):
  xT [C, B*T]  (host pre-transposes x)
  qT/kT/vT [128, B*T] = W_local^T @ x + b      (PE, fp32r)
  S^T tile [k 128, q 512] = K @ Q^T            (PE)  -- causal tiles only
  E^T = exp(S^T/8) * causal_mask               (ACT + DVE)
  y'^T [65, q 512] += [v | 1]^T @ E^T          (PE; row 64 = softmax sums)
  y_norm^T = y'^T[0:64] * bcast(1/sums)        (PE rank-1 bcast + DVE)
  partial^T [c 128, row 512] = Wp_local^T-ish  (PE) + b_proj  -> DRAM

fp32r is used for all matmuls (full PE rate at free-dim 512, ~1e-4 rel
err); operands are rounded to fp32r by their producing engine (gpsimd
casting DMAs for DRAM inputs, ACT/DVE for intermediates).
"""

import numpy as np

import concourse.bass as bass
import concourse.tile as tile
from concourse import bacc, mybir
from concourse.bass_utils import run_bass_kernel_spmd
from concourse.masks import make_identity

F32 = mybir.dt.float32
F32R = mybir.dt.float32r

B, T, C, H = 2, 2048, 1024, 16
HS = C // H            # 64 head dim
NCORES = 8
HL = H // NCORES       # 2 local heads
LC = HL * HS           # 128 local q/k/v cols
R = B * T              # 4096 rows (b, t)
KC = C // 128          # 8 contraction chunks for projections
QT = 512               # attention q tile (free dim)
NQT = T // QT          # 4
KA = 128               # attention k chunk (partition dim)
NKA = T // KA          # 16
RT = 512               # row tile for projections
NRT = R // RT          # 8
NCC = C // 128         # 8 c_proj output chunks


def build_program():
    nc = bacc.Bacc("TRN2", target_bir_lowering=False, debug=False,
                   num_devices=NCORES)

    xT = nc.dram_tensor("xT", [C, R], F32, kind="ExternalInput").ap()
    wqkv = nc.dram_tensor("wqkv", [C, 3 * LC], F32, kind="ExternalInput").ap()
    bqkv = nc.dram_tensor("bqkv", [3 * LC], F32, kind="ExternalInput").ap()
    wp = nc.dram_tensor("wp", [LC, C], F32, kind="ExternalInput").ap()
    bp = nc.dram_tensor("bp", [C], F32, kind="ExternalInput").ap()
    trimask = nc.dram_tensor("trimask", [KA, KA], F32, kind="ExternalInput").ap()
    outT = nc.dram_tensor("outT", [C, R], F32, kind="ExternalOutput").ap()

    with tile.TileContext(nc) as tc:
        with (
            tc.tile_pool(name="consts", bufs=1) as consts,
            tc.tile_pool(name="weights", bufs=1) as weights,
            tc.tile_pool(name="qkvT", bufs=1) as qkvT_pool,
            tc.tile_pool(name="xs", bufs=2) as xs_pool,
            tc.tile_pool(name="vp", bufs=2 * NKA) as vp_pool,
            tc.tile_pool(name="et", bufs=4) as et_pool,
            tc.tile_pool(name="ysb", bufs=2) as ysb_pool,
            tc.tile_pool(name="rec", bufs=2) as rec_pool,
            tc.tile_pool(name="osb", bufs=3) as osb_pool,
            tc.tile_pool(name="mm512", bufs=2, space="PSUM") as mm512_pool,
            tc.tile_pool(name="ytps", bufs=NQT, space="PSUM") as ytps_pool,
            tc.tile_pool(name="smps", bufs=2, space="PSUM") as smps_pool,
        ):
            # ---- constants ----
            identity = consts.tile([128, 128], F32)
            make_identity(nc, identity)
            ones64_f = consts.tile([1, HS], F32)
            nc.vector.memset(ones64_f, 1.0)
            ones64 = consts.tile([1, HS], F32R)
            nc.vector.tensor_copy(ones64, ones64_f)
            ones_col = consts.tile([128, 1], F32)
            nc.vector.memset(ones_col, 1.0)
            tri_sb = consts.tile([KA, KA], F32R)
            nc.gpsimd.dma_start(out=tri_sb, in_=trimask)
            bqkv_sb = consts.tile([128, 3], F32)
            nc.sync.dma_start(out=bqkv_sb, in_=bqkv.rearrange("(j p) -> p j", p=128))
            bp_sb = consts.tile([128, NCC], F32)
            nc.sync.dma_start(out=bp_sb, in_=bp.rearrange("(j p) -> p j", p=128))

            # ---- weights (rounded to fp32r via gpsimd casting DMA) ----
            wq_sb = weights.tile([128, KC, 3 * LC], F32R)
            nc.gpsimd.dma_start(
                out=wq_sb, in_=wqkv.rearrange("(kc p) n -> p kc n", p=128))
            wp_sb = weights.tile([LC, C], F32R)
            nc.gpsimd.dma_start(out=wp_sb, in_=wp)

            # ---- phase 1: QKV projection (transposed outputs) ----
            qT_s = qkvT_pool.tile([LC, R], F32R, tag="qT")
            kT_s = qkvT_pool.tile([LC, R], F32R, tag="kT")
            vT_s = qkvT_pool.tile([LC, R], F32R, tag="vT")
            dst_tiles = [qT_s, kT_s, vT_s]

            for rt in range(NRT):
                x_sb = xs_pool.tile([128, KC, RT], F32R)
                nc.gpsimd.dma_start(
                    out=x_sb,
                    in_=xT[:, rt * RT:(rt + 1) * RT].rearrange(
                        "(kc p) r -> p kc r", p=128))
                for col in range(3):
                    ps = mm512_pool.tile([128, RT], F32, tag="mm512")
                    for kc in range(KC):
                        nc.tensor.matmul(
                            ps,
                            wq_sb[:, kc, col * LC:(col + 1) * LC],
                            x_sb[:, kc, :],
                            start=(kc == 0),
                            stop=(kc == KC - 1),
                        )
                    # PSUM -> SBUF with per-partition bias add, rounding to f32r
                    nc.vector.tensor_scalar_add(
                        dst_tiles[col][:, rt * RT:(rt + 1) * RT],
                        ps,
                        bqkv_sb[:, col:col + 1],
                    )

            # ---- phase 2: attention per (b, h) ----
            ynT_s = qkvT_pool.tile([LC, R], F32R, tag="ynT")
            for b in range(B):
                base = b * T
                for h in range(HL):
                    hsl = slice(h * HS, (h + 1) * HS)
                    # v' tiles: [k 128, 64 v-cols | ones]
                    vps = []
                    for kc in range(NKA):
                        vp = vp_pool.tile([KA, HS + 1], F32R)
                        tp = smps_pool.tile([KA, HS], F32, tag="sm")
                        nc.tensor.transpose(
                            tp,
                            vT_s[hsl, base + kc * KA: base + (kc + 1) * KA]
                            .bitcast(F32),
                            identity[hsl, hsl],
                        )
                        nc.vector.tensor_copy(vp[:, 0:HS], tp)
                        nc.vector.tensor_copy(vp[:, HS:HS + 1], ones_col)
                        vps.append(vp)

                    yps = [
                        ytps_pool.tile([HS + 1, QT], F32, tag="yt",
                                       name=f"yt_b{b}h{h}q{qt}")
                        for qt in range(NQT)
                    ]
                    for kc in range(NKA):
                        qt_min = kc * KA // QT
                        for qt in range(qt_min, NQT):
                            sps = smps_pool.tile([KA, QT], F32, tag="sm")
                            nc.tensor.matmul(
                                sps,
                                kT_s[hsl, base + kc * KA: base + (kc + 1) * KA],
                                qT_s[hsl, base + qt * QT: base + (qt + 1) * QT],
                                start=True,
                                stop=True,
                            )
                            et = et_pool.tile([KA, QT], F32R)
                            # columns < off of a diagonal tile are fully
                            # masked; skip them entirely (the AV matmul
                            # accumulates only the [off, QT) span).
                            off = kc * KA - qt * QT if qt == qt_min else 0
                            nc.scalar.activation(
                                et[:, off:QT], sps[:, off:QT],
                                mybir.ActivationFunctionType.Exp,
                                scale=1.0 / np.sqrt(HS).item(),
                            )
                            if qt == qt_min:
                                # [off, off+128) is the triangular block
                                nc.vector.tensor_mul(
                                    et[:, off:off + KA],
                                    et[:, off:off + KA],
                                    tri_sb,
                                )
                            nc.tensor.matmul(
                                yps[qt][:, off:QT],
                                vps[kc],
                                et[:, off:QT],
                                start=(kc == 0),
                                stop=(kc == (qt + 1) * (QT // KA) - 1),
                            )

                    # normalize: y_norm^T = y'^T[0:64] * bcast(1 / sums)
                    for qt in range(NQT):
                        yts = ysb_pool.tile([HS + 1, QT], F32)
                        nc.vector.tensor_copy(yts, yps[qt])
                        rec = rec_pool.tile([1, QT], F32R)
                        with nc.allow_low_precision(
                                reason="fp32r reciprocal: ~1e-4 rel err ok"):
                            nc.vector.reciprocal(rec, yts[HS:HS + 1, :])
                        bc = smps_pool.tile([HS, QT], F32, tag="sm")
                        nc.tensor.matmul(bc, ones64, rec, start=True, stop=True)
                        nc.vector.tensor_mul(
                            ynT_s[hsl, base + qt * QT: base + (qt + 1) * QT],
                            yts[0:HS, :],
                            bc,
                        )

            # ---- phase 3: c_proj partial (transposed output) ----
            for cc in range(NCC):
                for rt in range(NRT):
                    pps = mm512_pool.tile([128, RT], F32, tag="mm512")
                    nc.tensor.matmul(
                        pps,
                        wp_sb[:, cc * 128:(cc + 1) * 128],
                        ynT_s[:, rt * RT:(rt + 1) * RT],
                        start=True,
                        stop=True,
                    )
                    o_sb = osb_pool.tile([128, RT], F32)
                    nc.vector.tensor_scalar_add(o_sb, pps, bp_sb[:, cc:cc + 1])
                    nc.sync.dma_start(
                        out=outT[cc * 128:(cc + 1) * 128,
                                 rt * RT:(rt + 1) * RT],
                        in_=o_sb,
                    )

    nc.compile()
    return nc


_NC = None


def _get_nc():
    global _NC
    if _NC is None:
        _NC = build_program()
    return _NC


def make_in_maps(x, W_attn, b_attn, W_proj, b_proj):
    x = np.asarray(x, np.float32)
    W_attn = np.asarray(W_attn, np.float32)
    b_attn = np.asarray(b_attn, np.float32)
    W_proj = np.asarray(W_proj, np.float32)
    b_proj = np.asarray(b_proj, np.float32)

    xT = np.ascontiguousarray(x.reshape(R, C).T)
    tri = np.triu(np.ones((KA, KA), np.float32))  # [kk, j]: 1 if j >= kk
    zeros_bp = np.zeros_like(b_proj)

    in_maps = []
    for core in range(NCORES):
        g0 = core * HL * HS  # first local column in head space
        cols = slice(g0, g0 + LC)
        w_local = np.concatenate(
            [W_attn[:, i * C:(i + 1) * C][:, cols] for i in range(3)], axis=1)
        b_local = np.concatenate(
            [b_attn[i * C:(i + 1) * C][cols] for i in range(3)])
        in_maps.append({
            "xT": xT,
            "wqkv": np.ascontiguousarray(w_local),
            "bqkv": np.ascontiguousarray(b_local),
            "wp": np.ascontiguousarray(W_proj[cols, :]),
            "bp": b_proj if core == 0 else zeros_bp,
            "trimask": tri,
        })
    return in_maps


def kernel(x, W_attn, b_attn, W_proj, b_proj):
    nc = _get_nc()
    in_maps = make_in_maps(x, W_attn, b_attn, W_proj, b_proj)
    res = run_bass_kernel_spmd(nc, in_maps, list(range(NCORES)))
    acc = res.results[0]["outT"].astype(np.float64)
    for corer in res.results[1:]:
        acc += corer["outT"]
    return np.ascontiguousarray(acc.T).reshape(B, T, C).astype(np.float32)


# revision 22
# speedup vs baseline: 1.4518x; 1.4518x over previous
"""Causal self-attention (B=2, T=2048, C=1024, H=16) on 8 Trainium2 cores.

Sharding: tensor-parallel over heads (2 heads/core). Each core computes
QKV projection for its heads, causal attention, and a partial c_proj
output; partials are summed on the host (b_proj is added by core 0 only).

Per-core dataflow (everything kept "K-major" so no activation transposes
are needed on the critical path):
  xT [C, B*T]  (host pre-transposes x)
  qT/kT/vT [128, B*T] = W_local^T @ x + b      (PE, fp32r)
  S^T tile [k 128, q 512] = K @ Q^T            (PE)  -- causal tiles only
  E^T = exp(S^T/8) * causal_mask               (ACT + DVE)
  y'^T [65, q 512] += [v | 1]^T @ E^T          (PE; row 64 = softmax sums)
  y_norm^T = y'^T[0:64] * bcast(1/sums)        (PE rank-1 bcast + DVE)
  partial^T [c 128, row 512] = Wp_local^T-ish  (PE) + b_proj  -> DRAM

fp32r is used for all matmuls (full PE rate at free-dim 512, ~1e-4 rel
err); operands are rounded to fp32r by their producing engine (gpsimd
casting DMAs for DRAM inputs, ACT/DVE for intermediates).
"""

import numpy as np

import concourse.bass as bass
import concourse.tile as tile
from concourse import bacc, mybir
from concourse.bass_utils import run_bass_kernel_spmd
from concourse.masks import make_identity

F32 = mybir.dt.float32
F32R = mybir.dt.float32r

B, T, C, H = 2, 2048, 1024, 16
HS = C // H            # 64 head dim
NCORES = 8
HL = H // NCORES       # 2 local heads
LC = HL * HS           # 128 local q/k/v cols
R = B * T              # 4096 rows (b, t)
KC = C // 128          # 8 contraction chunks for projections
QT = 512               # attention q tile (free dim)
NQT = T // QT          # 4
KA = 128               # attention k chunk (partition dim)
NKA = T // KA          # 16
RT = 512               # row tile for projections
NRT = R // RT          # 8
NCC = C // 128         # 8 c_proj output chunks


def build_program():
    nc = bacc.Bacc("TRN2", target_bir_lowering=False, debug=False,
                   num_devices=NCORES)

    xT = nc.dram_tensor("xT", [C, R], F32R, kind="ExternalInput").ap()
    wqkv = nc.dram_tensor("wqkv", [C, 3 * LC], F32R, kind="ExternalInput").ap()
    bqkv = nc.dram_tensor("bqkv", [3 * LC], F32, kind="ExternalInput").ap()
    wp = nc.dram_tensor("wp", [LC, C], F32R, kind="ExternalInput").ap()
    bp = nc.dram_tensor("bp", [C], F32, kind="ExternalInput").ap()
    trimask = nc.dram_tensor("trimask", [KA, KA], F32R, kind="ExternalInput").ap()
    outT = nc.dram_tensor("outT", [C, R], F32, kind="ExternalOutput").ap()

    with tile.TileContext(nc) as tc:
        with (
            tc.tile_pool(name="consts", bufs=1) as consts,
            tc.tile_pool(name="weights", bufs=1) as weights,
            tc.tile_pool(name="qkvT", bufs=1) as qkvT_pool,
            tc.tile_pool(name="xs", bufs=2) as xs_pool,
            tc.tile_pool(name="vp", bufs=2 * NKA) as vp_pool,
            tc.tile_pool(name="et", bufs=6) as et_pool,
            tc.tile_pool(name="ysb", bufs=2) as ysb_pool,
            tc.tile_pool(name="rec", bufs=2) as rec_pool,
            tc.tile_pool(name="osb", bufs=3) as osb_pool,
            tc.tile_pool(name="dscr", bufs=4, space="DRAM") as dscr_pool,
            tc.tile_pool(name="mm512", bufs=2, space="PSUM") as mm512_pool,
            tc.tile_pool(name="ytps", bufs=2, space="PSUM") as ytps_pool,
            tc.tile_pool(name="smps", bufs=3, space="PSUM") as smps_pool,
        ):
            # ---- constants ----
            identity = consts.tile([128, 128], F32)
            make_identity(nc, identity)
            ones64_f = consts.tile([1, HS], F32)
            nc.vector.memset(ones64_f, 1.0)
            ones64 = consts.tile([1, HS], F32R)
            nc.vector.tensor_copy(ones64, ones64_f)
            ones_col = consts.tile([128, 1], F32)
            nc.vector.memset(ones_col, 1.0)
            tri_sb = consts.tile([KA, KA], F32R)
            bqkv_sb = consts.tile([128, 3], F32)
            bp_sb = consts.tile([128, NCC], F32)

            # ---- weights (fp32r-typed DRAM, plain HWDGE loads) ----
            wq_sb = weights.tile([128, KC, 3 * LC], F32R)
            wq_r = wqkv.rearrange("(kc p) n -> p kc n", p=128)
            nc.sync.dma_start(out=wq_sb[:, 0:KC // 2], in_=wq_r[:, 0:KC // 2])
            nc.sync.dma_start(out=wq_sb[:, KC // 2:], in_=wq_r[:, KC // 2:])
            nc.sync.dma_start(
                out=bqkv_sb, in_=bqkv.rearrange("(j p) -> p j", p=128))
            wp_sb = weights.tile([LC, C], F32R)

            def load_consts():
                nc.sync.dma_start(out=tri_sb, in_=trimask)
                nc.sync.dma_start(
                    out=bp_sb, in_=bp.rearrange("(j p) -> p j", p=128))
                nc.sync.dma_start(out=wp_sb, in_=wp)

            # ---- phase 1: QKV projection (transposed outputs) ----
            qT_s = qkvT_pool.tile([LC, R], F32R, tag="qT")
            kT_s = qkvT_pool.tile([LC, R], F32R, tag="kT")
            vT_s = qkvT_pool.tile([LC, R], F32R, tag="vT")
            dst_tiles = [qT_s, kT_s, vT_s]

            def qkv_rowtile(rt):
                x_sb = xs_pool.tile([128, KC, RT], F32R, tag="xs", name=f"x_sb_rt{rt}")
                x_r = xT[:, rt * RT:(rt + 1) * RT].rearrange(
                    "(kc p) r -> p kc r", p=128)
                nc.scalar.dma_start(out=x_sb[:, 0:KC // 2],
                                    in_=x_r[:, 0:KC // 2])
                nc.scalar.dma_start(out=x_sb[:, KC // 2:],
                                    in_=x_r[:, KC // 2:])
                for col in range(3):
                    ps = mm512_pool.tile([128, RT], F32, tag="mm512",
                                         name=f"qkv_ps_rt{rt}c{col}")
                    for kc in range(KC):
                        nc.tensor.matmul(
                            ps,
                            wq_sb[:, kc, col * LC:(col + 1) * LC],
                            x_sb[:, kc, :],
                            start=(kc == 0),
                            stop=(kc == KC - 1),
                        )
                    # PSUM -> SBUF with per-partition bias add, rounding to f32r
                    nc.vector.tensor_scalar_add(
                        dst_tiles[col][:, rt * RT:(rt + 1) * RT],
                        ps,
                        bqkv_sb[:, col:col + 1],
                    )

            def proj_rowtile(rt, tail):
                """c_proj partial for row tile rt (needs ynT rows complete)."""
                half = RT // 2
                for cc in range(NCC):
                    if tail and cc % 2 == 1:
                        pps = smps_pool.tile([128, RT], F32, tag="sm",
                                             name=f"pps_rt{rt}c{cc}")
                    else:
                        pps = mm512_pool.tile([128, RT], F32, tag="mm512",
                                              name=f"pps_rt{rt}c{cc}")
                    nc.tensor.matmul(
                        pps,
                        wp_sb[:, cc * 128:(cc + 1) * 128],
                        ynT_s[:, rt * RT:(rt + 1) * RT],
                        start=True,
                        stop=True,
                    )
                    o_sb = osb_pool.tile([128, RT], F32, tag="osb",
                                         name=f"o_sb_rt{rt}c{cc}")
                    nc.scalar.activation(
                        o_sb[:, 0:half], pps[:, 0:half],
                        mybir.ActivationFunctionType.Identity,
                        bias=bp_sb[:, cc:cc + 1],
                    )
                    nc.vector.tensor_scalar_add(
                        o_sb[:, half:RT], pps[:, half:RT],
                        bp_sb[:, cc:cc + 1])
                    nc.sync.dma_start(
                        out=outT[cc * 128:(cc + 1) * 128,
                                 rt * RT:(rt + 1) * RT],
                        in_=o_sb,
                    )

            # ---- phase 2: attention per (b, h), interleaved with QKV/proj ----
            ynT_s = qkvT_pool.tile([LC, R], F32R, tag="ynT")
            qkv_rowtile(0)
            load_consts()
            for rt in range(1, NRT // 2):
                qkv_rowtile(rt)
            for b in range(B):
                base = b * T
                if b + 1 < B:
                    for rt in range((b + 1) * NRT // 2, (b + 2) * NRT // 2):
                        qkv_rowtile(rt)
                for h in range(HL):
                    hsl = slice(h * HS, (h + 1) * HS)
                    # v' tiles: [k 128, 64 v-cols | ones]
                    vps = []
                    for kc in range(NKA):
                        vp = vp_pool.tile([KA, HS + 1], F32R)
                        tp = smps_pool.tile([KA, HS], F32, tag="vt", bufs=1)
                        nc.tensor.transpose(
                            tp,
                            vT_s[hsl, base + kc * KA: base + (kc + 1) * KA]
                            .bitcast(F32),
                            identity[hsl, hsl],
                        )
                        nc.vector.tensor_copy(vp[:, 0:HS], tp)
                        nc.gpsimd.tensor_copy(vp[:, HS:HS + 1], ones_col)
                        vps.append(vp)

                    # qt-outer: only one y' accumulator live at a time
                    for qt in range(NQT):
                        yp = ytps_pool.tile([HS + 1, QT], F32, tag="yt",
                                            name=f"yt_b{b}h{h}q{qt}")
                        nka_q = (qt + 1) * (QT // KA)
                        for kc in range(nka_q):
                            diag = (kc * KA // QT == qt)
                            sps = smps_pool.tile(
                                [KA, QT], F32, tag="sm",
                                name=f"sps_b{b}h{h}q{qt}k{kc}")
                            nc.tensor.matmul(
                                sps,
                                kT_s[hsl,
                                     base + kc * KA: base + (kc + 1) * KA],
                                qT_s[hsl,
                                     base + qt * QT: base + (qt + 1) * QT],
                                start=True,
                                stop=True,
                            )
                            et = et_pool.tile([KA, QT], F32R, tag="et",
                                              name=f"et_b{b}h{h}q{qt}k{kc}")
                            # columns < off of a diagonal tile are fully
                            # masked; skip them entirely (the AV matmul
                            # accumulates only the [off, QT) span).
                            off = kc * KA - qt * QT if diag else 0
                            nc.scalar.activation(
                                et[:, off:QT], sps[:, off:QT],
                                mybir.ActivationFunctionType.Exp,
                                scale=1.0 / np.sqrt(HS).item(),
                            )
                            if diag:
                                # [off, off+128) is the triangular block
                                nc.gpsimd.tensor_mul(
                                    et[:, off:off + KA],
                                    et[:, off:off + KA],
                                    tri_sb,
                                )
                            nc.tensor.matmul(
                                yp[:, off:QT],
                                vps[kc],
                                et[:, off:QT],
                                start=(kc == 0),
                                stop=(kc == nka_q - 1),
                            )

                        # normalize: y_norm^T = y'^T[0:64] * bcast(1 / sums)
                        yts = ysb_pool.tile([HS + 1, QT], F32, tag="yts",
                                            name=f"yts_b{b}h{h}q{qt}")
                        nc.vector.tensor_copy(yts, yp)
                        rec = rec_pool.tile([1, QT], F32R, tag="rec",
                                            name=f"rec_b{b}h{h}q{qt}")
                        with nc.allow_low_precision(
                                reason="fp32r reciprocal: ~1e-4 rel err ok"):
                            nc.vector.reciprocal(rec, yts[HS:HS + 1, :])
                        bcs = ysb_pool.tile([HS, QT], F32R, tag="bcs",
                                            name=f"bcs_b{b}h{h}q{qt}")
                        recd = dscr_pool.tile([1, QT], F32R, tag="recd",
                                              name=f"recd_b{b}h{h}q{qt}")
                        nc.sync.dma_start(out=recd, in_=rec)
                        rec_bcast = bass.AP(
                            tensor=recd.tensor, offset=recd.offset,
                            ap=[[0, HS]] + [list(d) for d in recd.ap[1:]])
                        nc.sync.dma_start(out=bcs, in_=rec_bcast)
                        nc.vector.tensor_mul(
                            ynT_s[hsl, base + qt * QT: base + (qt + 1) * QT],
                            yts[0:HS, :],
                            bcs,
                        )
                        # c_proj row tiles interleave into the last head's
                        # attention, one qt behind the normalize that feeds
                        # them, so the PE never waits on the bcast chain and
                        # output DMA spreads across the attention window.
                        if h == HL - 1 and qt > 0:
                            proj_rowtile(b * NRT // 2 + qt - 1, tail=False)

                # last row tile of this batch after its attention finishes
                proj_rowtile(b * NRT // 2 + NQT - 1, tail=(b == B - 1))

    nc.compile()
    return nc


_NC = None


def _get_nc():
    global _NC
    if _NC is None:
        _NC = build_program()
    return _NC


def make_in_maps(x, W_attn, b_attn, W_proj, b_proj):
    x = np.asarray(x, np.float32)
    W_attn = np.asarray(W_attn, np.float32)
    b_attn = np.asarray(b_attn, np.float32)
    W_proj = np.asarray(W_proj, np.float32)
    b_proj = np.asarray(b_proj, np.float32)

    xT = np.ascontiguousarray(x.reshape(R, C).T)
    tri = np.triu(np.ones((KA, KA), np.float32))  # [kk, j]: 1 if j >= kk
    zeros_bp = np.zeros_like(b_proj)

    in_maps = []
    for core in range(NCORES):
        g0 = core * HL * HS  # first local column in head space
        cols = slice(g0, g0 + LC)
        w_local = np.concatenate(
            [W_attn[:, i * C:(i + 1) * C][:, cols] for i in range(3)], axis=1)
        b_local = np.concatenate(
            [b_attn[i * C:(i + 1) * C][cols] for i in range(3)])
        in_maps.append({
            "xT": xT,
            "wqkv": np.ascontiguousarray(w_local),
            "bqkv": np.ascontiguousarray(b_local),
            "wp": np.ascontiguousarray(W_proj[cols, :]),
            "bp": b_proj if core == 0 else zeros_bp,
            "trimask": tri,
        })
    return in_maps


def kernel(x, W_attn, b_attn, W_proj, b_proj):
    nc = _get_nc()
    in_maps = make_in_maps(x, W_attn, b_attn, W_proj, b_proj)
    res = run_bass_kernel_spmd(nc, in_maps, list(range(NCORES)))
    acc = res.results[0]["outT"].astype(np.float64)
    for corer in res.results[1:]:
        acc += corer["outT"]
    return np.ascontiguousarray(acc.T).reshape(B, T, C).astype(np.float32)


# revision 26
# speedup vs baseline: 1.4930x; 1.0284x over previous
"""Causal self-attention (B=2, T=2048, C=1024, H=16) on 8 Trainium2 cores.

Sharding: tensor-parallel over heads (2 heads/core). Each core computes
QKV projection for its heads, causal attention, and a partial c_proj
output; partials are summed on the host (b_proj is added by core 0 only).

Per-core dataflow (everything kept "K-major" so no activation transposes
are needed on the critical path):
  xT [C, B*T]  (host pre-transposes x)
  qT/kT/vT [128, B*T] = W_local^T @ x + b      (PE, fp32r)
  S^T tile [k 128, q 512] = K @ Q^T            (PE)  -- causal tiles only
  E^T = exp(S^T/8) * causal_mask               (ACT + DVE)
  y'^T [65, q 512] += [v | 1]^T @ E^T          (PE; row 64 = softmax sums)
  y_norm^T = y'^T[0:64] * bcast(1/sums)        (PE rank-1 bcast + DVE)
  partial^T [c 128, row 512] = Wp_local^T-ish  (PE) + b_proj  -> DRAM

fp32r is used for all matmuls (full PE rate at free-dim 512, ~1e-4 rel
err); operands are rounded to fp32r by their producing engine (gpsimd
casting DMAs for DRAM inputs, ACT/DVE for intermediates).
"""

import numpy as np

import concourse.bass as bass
import concourse.tile as tile
from concourse import bacc, mybir
from concourse.bass_utils import run_bass_kernel_spmd
from concourse.masks import make_identity

F32 = mybir.dt.float32
F32R = mybir.dt.float32r

B, T, C, H = 2, 2048, 1024, 16
HS = C // H            # 64 head dim
NCORES = 8
HL = H // NCORES       # 2 local heads
LC = HL * HS           # 128 local q/k/v cols
R = B * T              # 4096 rows (b, t)
KC = C // 128          # 8 contraction chunks for projections
QT = 512               # attention q tile (free dim)
NQT = T // QT          # 4
KA = 128               # attention k chunk (partition dim)
NKA = T // KA          # 16
RT = 512               # row tile for projections
NRT = R // RT          # 8
NCC = C // 128         # 8 c_proj output chunks


def build_program():
    nc = bacc.Bacc("TRN2", target_bir_lowering=False, debug=False,
                   num_devices=NCORES)

    xT = nc.dram_tensor("xT", [C, R], F32R, kind="ExternalInput").ap()
    wqkv = nc.dram_tensor("wqkv", [C, 3 * LC], F32R, kind="ExternalInput").ap()
    bqkv = nc.dram_tensor("bqkv", [3 * LC], F32, kind="ExternalInput").ap()
    wp = nc.dram_tensor("wp", [LC, C], F32R, kind="ExternalInput").ap()
    bp = nc.dram_tensor("bp", [C], F32, kind="ExternalInput").ap()
    trimask = nc.dram_tensor("trimask", [KA, KA], F32R, kind="ExternalInput").ap()
    outT = nc.dram_tensor("outT", [C, R], F32, kind="ExternalOutput").ap()

    with tile.TileContext(nc) as tc:
        with (
            tc.tile_pool(name="consts", bufs=1) as consts,
            tc.tile_pool(name="weights", bufs=1) as weights,
            tc.tile_pool(name="qkvT", bufs=1) as qkvT_pool,
            tc.tile_pool(name="xs", bufs=2) as xs_pool,
            tc.tile_pool(name="vp", bufs=2 * NKA) as vp_pool,
            tc.tile_pool(name="et", bufs=6) as et_pool,
            tc.tile_pool(name="ysb", bufs=2) as ysb_pool,
            tc.tile_pool(name="rec", bufs=2) as rec_pool,
            tc.tile_pool(name="osb", bufs=6) as osb_pool,
            tc.tile_pool(name="dscr", bufs=4, space="DRAM") as dscr_pool,
            tc.tile_pool(name="mm512", bufs=2, space="PSUM") as mm512_pool,
            tc.tile_pool(name="ytps", bufs=2, space="PSUM") as ytps_pool,
            tc.tile_pool(name="smps", bufs=3, space="PSUM") as smps_pool,
        ):
            # ---- constants ----
            identity = consts.tile([128, 128], F32)
            make_identity(nc, identity)
            ones64_f = consts.tile([1, HS], F32)
            nc.vector.memset(ones64_f, 1.0)
            ones64 = consts.tile([1, HS], F32R)
            nc.vector.tensor_copy(ones64, ones64_f)
            ones_col = consts.tile([128, 1], F32)
            nc.vector.memset(ones_col, 1.0)
            tri_sb = consts.tile([KA, KA], F32R)
            bqkv_sb = consts.tile([128, 3], F32)
            bp_sb = consts.tile([128, NCC], F32)

            # ---- weights (fp32r-typed DRAM, plain HWDGE loads) ----
            wq_sb = weights.tile([128, KC, 3 * LC], F32R)
            wq_r = wqkv.rearrange("(kc p) n -> p kc n", p=128)
            nc.sync.dma_start(out=wq_sb[:, 0:2], in_=wq_r[:, 0:2])
            nc.sync.dma_start(out=wq_sb[:, 2:KC], in_=wq_r[:, 2:KC])
            nc.sync.dma_start(
                out=bqkv_sb, in_=bqkv.rearrange("(j p) -> p j", p=128))
            wp_sb = weights.tile([LC, C], F32R)

            def load_consts():
                nc.sync.dma_start(out=tri_sb, in_=trimask)
                nc.sync.dma_start(
                    out=bp_sb, in_=bp.rearrange("(j p) -> p j", p=128))
                nc.sync.dma_start(out=wp_sb, in_=wp)

            # ---- phase 1: QKV projection (transposed outputs) ----
            qT_s = qkvT_pool.tile([LC, R], F32R, tag="qT")
            kT_s = qkvT_pool.tile([LC, R], F32R, tag="kT")
            vT_s = qkvT_pool.tile([LC, R], F32R, tag="vT")
            dst_tiles = [qT_s, kT_s, vT_s]

            def qkv_rowtile(rt):
                x_sb = xs_pool.tile([128, KC, RT], F32R, tag="xs", name=f"x_sb_rt{rt}")
                x_r = xT[:, rt * RT:(rt + 1) * RT].rearrange(
                    "(kc p) r -> p kc r", p=128)
                first = 2 if rt == 0 else KC // 2
                nc.scalar.dma_start(out=x_sb[:, 0:first], in_=x_r[:, 0:first])
                nc.scalar.dma_start(out=x_sb[:, first:KC],
                                    in_=x_r[:, first:KC])
                for col in range(3):
                    ps = mm512_pool.tile([128, RT], F32, tag="mm512",
                                         name=f"qkv_ps_rt{rt}c{col}")
                    for kc in range(KC):
                        nc.tensor.matmul(
                            ps,
                            wq_sb[:, kc, col * LC:(col + 1) * LC],
                            x_sb[:, kc, :],
                            start=(kc == 0),
                            stop=(kc == KC - 1),
                        )
                    # PSUM -> SBUF with per-partition bias add, rounding to f32r
                    nc.vector.tensor_scalar_add(
                        dst_tiles[col][:, rt * RT:(rt + 1) * RT],
                        ps,
                        bqkv_sb[:, col:col + 1],
                    )

            def proj_rowtile(rt, tail):
                """c_proj partial for row tile rt (needs ynT rows complete)."""
                half = RT // 2
                for cc in range(NCC):
                    if tail and cc % 2 == 1:
                        pps = smps_pool.tile([128, RT], F32, tag="sm",
                                             name=f"pps_rt{rt}c{cc}")
                    else:
                        pps = mm512_pool.tile([128, RT], F32, tag="mm512",
                                              name=f"pps_rt{rt}c{cc}")
                    nc.tensor.matmul(
                        pps,
                        wp_sb[:, cc * 128:(cc + 1) * 128],
                        ynT_s[:, rt * RT:(rt + 1) * RT],
                        start=True,
                        stop=True,
                    )
                    o_sb = osb_pool.tile([128, RT], F32, tag="osb",
                                         name=f"o_sb_rt{rt}c{cc}")
                    nc.scalar.activation(
                        o_sb[:, 0:half], pps[:, 0:half],
                        mybir.ActivationFunctionType.Identity,
                        bias=bp_sb[:, cc:cc + 1],
                    )
                    nc.vector.tensor_scalar_add(
                        o_sb[:, half:RT], pps[:, half:RT],
                        bp_sb[:, cc:cc + 1])
                    nc.sync.dma_start(
                        out=outT[cc * 128:(cc + 1) * 128,
                                 rt * RT:(rt + 1) * RT],
                        in_=o_sb,
                    )

            # ---- phase 2: attention per (b, h), interleaved with QKV/proj ----
            ynT_s = qkvT_pool.tile([LC, R], F32R, tag="ynT")
            qkv_rowtile(0)
            load_consts()
            for rt in range(1, NRT // 2):
                qkv_rowtile(rt)
            for b in range(B):
                base = b * T
                if b + 1 < B:
                    for rt in range((b + 1) * NRT // 2, (b + 2) * NRT // 2):
                        qkv_rowtile(rt)
                for h in range(HL):
                    hsl = slice(h * HS, (h + 1) * HS)
                    # v' tiles: [k 128, 64 v-cols | ones]
                    vps = []
                    for kc in range(NKA):
                        vp = vp_pool.tile([KA, HS + 1], F32R)
                        tp = smps_pool.tile([KA, HS], F32, tag="vt", bufs=1)
                        nc.tensor.transpose(
                            tp,
                            vT_s[hsl, base + kc * KA: base + (kc + 1) * KA]
                            .bitcast(F32),
                            identity[hsl, hsl],
                        )
                        nc.vector.tensor_copy(vp[:, 0:HS], tp)
                        nc.gpsimd.tensor_copy(vp[:, HS:HS + 1], ones_col)
                        vps.append(vp)

                    # qt-outer: only one y' accumulator live at a time
                    for qt in range(NQT):
                        yp = ytps_pool.tile([HS + 1, QT], F32, tag="yt",
                                            name=f"yt_b{b}h{h}q{qt}")
                        nka_q = (qt + 1) * (QT // KA)
                        for kc in range(nka_q):
                            diag = (kc * KA // QT == qt)
                            sps = smps_pool.tile(
                                [KA, QT], F32, tag="sm",
                                name=f"sps_b{b}h{h}q{qt}k{kc}")
                            nc.tensor.matmul(
                                sps,
                                kT_s[hsl,
                                     base + kc * KA: base + (kc + 1) * KA],
                                qT_s[hsl,
                                     base + qt * QT: base + (qt + 1) * QT],
                                start=True,
                                stop=True,
                            )
                            et = et_pool.tile([KA, QT], F32R, tag="et",
                                              name=f"et_b{b}h{h}q{qt}k{kc}")
                            # columns < off of a diagonal tile are fully
                            # masked; skip them entirely (the AV matmul
                            # accumulates only the [off, QT) span).
                            off = kc * KA - qt * QT if diag else 0
                            nc.scalar.activation(
                                et[:, off:QT], sps[:, off:QT],
                                mybir.ActivationFunctionType.Exp,
                                scale=1.0 / np.sqrt(HS).item(),
                            )
                            if diag:
                                # [off, off+128) is the triangular block
                                nc.gpsimd.tensor_mul(
                                    et[:, off:off + KA],
                                    et[:, off:off + KA],
                                    tri_sb,
                                )
                            nc.tensor.matmul(
                                yp[:, off:QT],
                                vps[kc],
                                et[:, off:QT],
                                start=(kc == 0),
                                stop=(kc == nka_q - 1),
                            )

                        # normalize: y_norm^T = y'^T[0:64] * bcast(1 / sums)
                        yts = ysb_pool.tile([HS + 1, QT], F32, tag="yts",
                                            name=f"yts_b{b}h{h}q{qt}")
                        nc.vector.tensor_copy(yts, yp)
                        rec = rec_pool.tile([1, QT], F32R, tag="rec",
                                            name=f"rec_b{b}h{h}q{qt}")
                        with nc.allow_low_precision(
                                reason="fp32r reciprocal: ~1e-4 rel err ok"):
                            nc.vector.reciprocal(rec, yts[HS:HS + 1, :])
                        bcs = ysb_pool.tile([HS, QT], F32R, tag="bcs",
                                            name=f"bcs_b{b}h{h}q{qt}")
                        recd = dscr_pool.tile([1, QT], F32R, tag="recd",
                                              name=f"recd_b{b}h{h}q{qt}")
                        nc.sync.dma_start(out=recd, in_=rec)
                        rec_bcast = bass.AP(
                            tensor=recd.tensor, offset=recd.offset,
                            ap=[[0, HS]] + [list(d) for d in recd.ap[1:]])
                        nc.sync.dma_start(out=bcs, in_=rec_bcast)
                        nc.vector.tensor_mul(
                            ynT_s[hsl, base + qt * QT: base + (qt + 1) * QT],
                            yts[0:HS, :],
                            bcs,
                        )
                        # c_proj row tiles interleave into the last head's
                        # attention, one qt behind the normalize that feeds
                        # them, so the PE never waits on the bcast chain and
                        # output DMA spreads across the attention window.
                        if h == HL - 1 and qt > 0:
                            proj_rowtile(b * NRT // 2 + qt - 1,
                                         tail=(b == B - 1 and qt == NQT - 1))

                # last row tile of this batch after its attention finishes
                proj_rowtile(b * NRT // 2 + NQT - 1, tail=(b == B - 1))

    nc.compile()
    return nc


_NC = None


def _get_nc():
    global _NC
    if _NC is None:
        _NC = build_program()
    return _NC


def make_in_maps(x, W_attn, b_attn, W_proj, b_proj):
    x = np.asarray(x, np.float32)
    W_attn = np.asarray(W_attn, np.float32)
    b_attn = np.asarray(b_attn, np.float32)
    W_proj = np.asarray(W_proj, np.float32)
    b_proj = np.asarray(b_proj, np.float32)

    xT = np.ascontiguousarray(x.reshape(R, C).T)
    tri = np.triu(np.ones((KA, KA), np.float32))  # [kk, j]: 1 if j >= kk
    zeros_bp = np.zeros_like(b_proj)

    in_maps = []
    for core in range(NCORES):
        g0 = core * HL * HS  # first local column in head space
        cols = slice(g0, g0 + LC)
        w_local = np.concatenate(
            [W_attn[:, i * C:(i + 1) * C][:, cols] for i in range(3)], axis=1)
        b_local = np.concatenate(
            [b_attn[i * C:(i + 1) * C][cols] for i in range(3)])
        in_maps.append({
            "xT": xT,
            "wqkv": np.ascontiguousarray(w_local),
            "bqkv": np.ascontiguousarray(b_local),
            "wp": np.ascontiguousarray(W_proj[cols, :]),
            "bp": b_proj if core == 0 else zeros_bp,
            "trimask": tri,
        })
    return in_maps


def kernel(x, W_attn, b_attn, W_proj, b_proj):
    nc = _get_nc()
    in_maps = make_in_maps(x, W_attn, b_attn, W_proj, b_proj)
    res = run_bass_kernel_spmd(nc, in_maps, list(range(NCORES)))
    acc = res.results[0]["outT"].astype(np.float64)
    for corer in res.results[1:]:
        acc += corer["outT"]
    return np.ascontiguousarray(acc.T).reshape(B, T, C).astype(np.float32)


# revision 33
# speedup vs baseline: 1.5467x; 1.0360x over previous
"""Causal self-attention (B=2, T=2048, C=1024, H=16) on 8 Trainium2 cores.

Sharding: tensor-parallel over heads (2 heads/core). Each core computes
QKV projection for its heads, causal attention, and a partial c_proj
output; partials are summed on the host (b_proj is added by core 0 only).

Per-core dataflow (everything kept "K-major" so no activation transposes
are needed on the critical path):
  xT [C, B*T]  (host pre-transposes x)
  qT/kT/vT [128, B*T] = W_local^T @ x + b      (PE, fp32r)
  S^T tile [k 128, q 512] = K @ Q^T            (PE)  -- causal tiles only
  E^T = exp(S^T/8) * causal_mask               (ACT + DVE)
  y'^T [65, q 512] += [v | 1]^T @ E^T          (PE; row 64 = softmax sums)
  y_norm^T = y'^T[0:64] * bcast(1/sums)        (PE rank-1 bcast + DVE)
  partial^T [c 128, row 512] = Wp_local^T-ish  (PE) + b_proj  -> DRAM

fp32r is used for all matmuls (full PE rate at free-dim 512, ~1e-4 rel
err); operands are rounded to fp32r by their producing engine (gpsimd
casting DMAs for DRAM inputs, ACT/DVE for intermediates).
"""

import numpy as np

import concourse.bass as bass
import concourse.tile as tile
from concourse import bacc, mybir
from concourse.bass_utils import run_bass_kernel_spmd
from concourse.masks import make_identity

F32 = mybir.dt.float32
F32R = mybir.dt.float32r

B, T, C, H = 2, 2048, 1024, 16
HS = C // H            # 64 head dim
NCORES = 8
HL = H // NCORES       # 2 local heads
LC = HL * HS           # 128 local q/k/v cols
R = B * T              # 4096 rows (b, t)
KC = C // 128          # 8 contraction chunks for projections
QT = 512               # attention q tile (free dim)
NQT = T // QT          # 4
KA = 128               # attention k chunk (partition dim)
NKA = T // KA          # 16
RT = 512               # row tile for projections
NRT = R // RT          # 8
NCC = C // 128         # 8 c_proj output chunks


def build_program():
    nc = bacc.Bacc("TRN2", target_bir_lowering=False, debug=False,
                   num_devices=NCORES)

    xT = nc.dram_tensor("xT", [C, R], F32R, kind="ExternalInput").ap()
    wqkv = nc.dram_tensor("wqkv", [C, 3 * LC], F32R, kind="ExternalInput").ap()
    bqkv = nc.dram_tensor("bqkv", [3 * LC], F32, kind="ExternalInput").ap()
    wp = nc.dram_tensor("wp", [LC, C], F32R, kind="ExternalInput").ap()
    bp = nc.dram_tensor("bp", [C], F32, kind="ExternalInput").ap()
    trimask = nc.dram_tensor("trimask", [KA, KA], F32R, kind="ExternalInput").ap()
    outT = nc.dram_tensor("outT", [C, R], F32, kind="ExternalOutput").ap()

    with tile.TileContext(nc) as tc:
        with (
            tc.tile_pool(name="consts", bufs=1) as consts,
            tc.tile_pool(name="weights", bufs=1) as weights,
            tc.tile_pool(name="qkvT", bufs=1) as qkvT_pool,
            tc.tile_pool(name="xs", bufs=3) as xs_pool,
            tc.tile_pool(name="vp", bufs=2 * NKA) as vp_pool,
            tc.tile_pool(name="et", bufs=8) as et_pool,
            tc.tile_pool(name="ysb", bufs=2) as ysb_pool,
            tc.tile_pool(name="rec", bufs=2) as rec_pool,
            tc.tile_pool(name="osb", bufs=6) as osb_pool,
            tc.tile_pool(name="dscr", bufs=4, space="DRAM") as dscr_pool,
            tc.tile_pool(name="mm512", bufs=3, space="PSUM") as mm512_pool,
            tc.tile_pool(name="ytps", bufs=2, space="PSUM") as ytps_pool,
            tc.tile_pool(name="smps", bufs=3, space="PSUM") as smps_pool,
        ):
            # ---- constants ----
            identity = consts.tile([128, 128], F32)
            make_identity(nc, identity)
            ones64_f = consts.tile([1, HS], F32)
            nc.vector.memset(ones64_f, 1.0)
            ones64 = consts.tile([1, HS], F32R)
            nc.vector.tensor_copy(ones64, ones64_f)
            ones_col = consts.tile([128, 1], F32)
            nc.vector.memset(ones_col, 1.0)
            tri_sb = consts.tile([KA, KA], F32R)
            bqkv_sb = consts.tile([128, 3], F32)
            bp_sb = consts.tile([128, NCC], F32)

            # ---- weights (fp32r-typed DRAM, plain HWDGE loads) ----
            wq_sb = weights.tile([128, KC, 3 * LC], F32R)
            wq_r = wqkv.rearrange("(kc p) n -> p kc n", p=128)
            nc.sync.dma_start(out=wq_sb[:, 0:2], in_=wq_r[:, 0:2])
            nc.sync.dma_start(
                out=bqkv_sb, in_=bqkv.rearrange("(j p) -> p j", p=128))
            wp_sb = weights.tile([LC, C], F32R)

            def load_consts():
                nc.sync.dma_start(out=tri_sb, in_=trimask)
                nc.sync.dma_start(
                    out=bp_sb, in_=bp.rearrange("(j p) -> p j", p=128))
                nc.sync.dma_start(out=wp_sb, in_=wp)

            # ---- phase 1: QKV projection (transposed outputs) ----
            qT_s = qkvT_pool.tile([LC, R], F32R, tag="qT")
            kT_s = qkvT_pool.tile([LC, R], F32R, tag="kT")
            vT_s = qkvT_pool.tile([LC, R], F32R, tag="vT")
            dst_tiles = [qT_s, kT_s, vT_s]

            def qkv_load(rt):
                x_sb = xs_pool.tile([128, KC, RT], F32R, tag="xs", name=f"x_sb_rt{rt}")
                x_r = xT[:, rt * RT:(rt + 1) * RT].rearrange(
                    "(kc p) r -> p kc r", p=128)
                first = 2 if rt == 0 else KC // 2
                nc.scalar.dma_start(out=x_sb[:, 0:first], in_=x_r[:, 0:first])
                nc.scalar.dma_start(out=x_sb[:, first:KC],
                                    in_=x_r[:, first:KC])
                return x_sb

            def qkv_compute(rt, x_sb):
                for col in range(3):
                    ps = mm512_pool.tile([128, RT], F32, tag="mm512",
                                         name=f"qkv_ps_rt{rt}c{col}")
                    for kc in range(KC):
                        nc.tensor.matmul(
                            ps,
                            wq_sb[:, kc, col * LC:(col + 1) * LC],
                            x_sb[:, kc, :],
                            start=(kc == 0),
                            stop=(kc == KC - 1),
                        )
                    # PSUM -> SBUF with per-partition bias add, rounding to f32r
                    nc.vector.tensor_scalar_add(
                        dst_tiles[col][:, rt * RT:(rt + 1) * RT],
                        ps,
                        bqkv_sb[:, col:col + 1],
                    )

            def proj_rowtile(rt, tail):
                """c_proj partial for row tile rt (needs ynT rows complete)."""
                half = RT // 2
                for cc in range(NCC):
                    if tail and cc % 2 == 1:
                        pps = smps_pool.tile([128, RT], F32, tag="sm",
                                             name=f"pps_rt{rt}c{cc}")
                    else:
                        pps = mm512_pool.tile([128, RT], F32, tag="mm512",
                                              name=f"pps_rt{rt}c{cc}")
                    nc.tensor.matmul(
                        pps,
                        wp_sb[:, cc * 128:(cc + 1) * 128],
                        ynT_s[:, rt * RT:(rt + 1) * RT],
                        start=True,
                        stop=True,
                    )
                    o_sb = osb_pool.tile([128, RT], F32, tag="osb",
                                         name=f"o_sb_rt{rt}c{cc}")
                    nc.scalar.activation(
                        o_sb[:, 0:half], pps[:, 0:half],
                        mybir.ActivationFunctionType.Identity,
                        bias=bp_sb[:, cc:cc + 1],
                    )
                    nc.vector.tensor_scalar_add(
                        o_sb[:, half:RT], pps[:, half:RT],
                        bp_sb[:, cc:cc + 1])
                    nc.sync.dma_start(
                        out=outT[cc * 128:(cc + 1) * 128,
                                 rt * RT:(rt + 1) * RT],
                        in_=o_sb,
                    )

            # ---- phase 2: attention per (b, h), interleaved with QKV/proj ----
            ynT_s = qkvT_pool.tile([LC, R], F32R, tag="ynT")
            x0 = qkv_load(0)
            nc.sync.dma_start(out=wq_sb[:, 2:KC], in_=wq_r[:, 2:KC])
            qkv_compute(0, x0)
            x1 = qkv_load(1)
            load_consts()
            qkv_compute(1, x1)
            for rt in range(2, NRT // 2):
                qkv_compute(rt, qkv_load(rt))
            for b in range(B):
                base = b * T
                if b + 1 < B:
                    for rt in range((b + 1) * NRT // 2, (b + 2) * NRT // 2):
                        qkv_compute(rt, qkv_load(rt))
                for h in range(HL):
                    hsl = slice(h * HS, (h + 1) * HS)
                    # v' tiles: [k 128, 64 v-cols | ones]
                    vps = []
                    for kc in range(NKA):
                        vp = vp_pool.tile([KA, HS + 1], F32R)
                        tp = mm512_pool.tile([KA, HS], F32, tag="mm512")
                        nc.tensor.transpose(
                            tp,
                            vT_s[hsl, base + kc * KA: base + (kc + 1) * KA]
                            .bitcast(F32),
                            identity[hsl, hsl],
                        )
                        nc.vector.tensor_copy(vp[:, 0:HS], tp)
                        nc.gpsimd.tensor_copy(vp[:, HS:HS + 1], ones_col)
                        vps.append(vp)

                    # qt-outer: only one y' accumulator live at a time
                    for qt in range(NQT):
                        yp = ytps_pool.tile([HS + 1, QT], F32, tag="yt",
                                            name=f"yt_b{b}h{h}q{qt}")
                        nka_q = (qt + 1) * (QT // KA)
                        for kc in range(nka_q):
                            diag = (kc * KA // QT == qt)
                            sps = smps_pool.tile(
                                [KA, QT], F32, tag="sm",
                                name=f"sps_b{b}h{h}q{qt}k{kc}")
                            nc.tensor.matmul(
                                sps,
                                kT_s[hsl,
                                     base + kc * KA: base + (kc + 1) * KA],
                                qT_s[hsl,
                                     base + qt * QT: base + (qt + 1) * QT],
                                start=True,
                                stop=True,
                            )
                            et = et_pool.tile([KA, QT], F32R, tag="et",
                                              name=f"et_b{b}h{h}q{qt}k{kc}")
                            # columns < off of a diagonal tile are fully
                            # masked; skip them entirely (the AV matmul
                            # accumulates only the [off, QT) span).
                            off = kc * KA - qt * QT if diag else 0
                            nc.scalar.activation(
                                et[:, off:QT], sps[:, off:QT],
                                mybir.ActivationFunctionType.Exp,
                                scale=1.0 / np.sqrt(HS).item(),
                            )
                            if diag:
                                # [off, off+128) is the triangular block
                                nc.gpsimd.tensor_mul(
                                    et[:, off:off + KA],
                                    et[:, off:off + KA],
                                    tri_sb,
                                )
                            nc.tensor.matmul(
                                yp[:, off:QT],
                                vps[kc],
                                et[:, off:QT],
                                start=(kc == 0),
                                stop=(kc == nka_q - 1),
                            )

                        # normalize: y_norm^T = y'^T[0:64] * bcast(1 / sums)
                        yts = ysb_pool.tile([HS + 1, QT], F32, tag="yts",
                                            name=f"yts_b{b}h{h}q{qt}")
                        nc.vector.tensor_copy(yts, yp)
                        rec = rec_pool.tile([1, QT], F32R, tag="rec",
                                            name=f"rec_b{b}h{h}q{qt}")
                        with nc.allow_low_precision(
                                reason="fp32r reciprocal: ~1e-4 rel err ok"):
                            nc.vector.reciprocal(rec, yts[HS:HS + 1, :])
                        if qt == NQT - 1 and h == HL - 1:
                            # end of batch: nothing else keeps PE busy, and
                            # the DRAM-bounce latency would gate the final
                            # c_proj row tile -- use a rank-1 PE broadcast
                            bcp = smps_pool.tile([HS, QT], F32, tag="sm",
                                                 name=f"bcp_b{b}h{h}q{qt}")
                            nc.tensor.matmul(bcp, ones64, rec,
                                             start=True, stop=True)
                            nc.vector.tensor_mul(
                                ynT_s[hsl,
                                      base + qt * QT: base + (qt + 1) * QT],
                                yts[0:HS, :],
                                bcp,
                            )
                        else:
                            bcs = ysb_pool.tile([HS, QT], F32R, tag="bcs",
                                                name=f"bcs_b{b}h{h}q{qt}")
                            recd = dscr_pool.tile([1, QT], F32R, tag="recd",
                                                  name=f"recd_b{b}h{h}q{qt}")
                            nc.sync.dma_start(out=recd, in_=rec)
                            rec_bcast = bass.AP(
                                tensor=recd.tensor, offset=recd.offset,
                                ap=[[0, HS]] + [list(d) for d in recd.ap[1:]])
                            nc.sync.dma_start(out=bcs, in_=rec_bcast)
                            nc.vector.tensor_mul(
                                ynT_s[hsl,
                                      base + qt * QT: base + (qt + 1) * QT],
                                yts[0:HS, :],
                                bcs,
                            )
                        # c_proj row tiles interleave into the last head's
                        # attention, one qt behind the normalize that feeds
                        # them, so the PE never waits on the bcast chain and
                        # output DMA spreads across the attention window.
                        if h == HL - 1 and qt > 0:
                            proj_rowtile(b * NRT // 2 + qt - 1,
                                         tail=(b == B - 1 and qt == NQT - 1))

                # last row tile of this batch after its attention finishes
                proj_rowtile(b * NRT // 2 + NQT - 1, tail=(b == B - 1))

    nc.compile()
    return nc


_NC = None


def _get_nc():
    global _NC
    if _NC is None:
        _NC = build_program()
    return _NC


def make_in_maps(x, W_attn, b_attn, W_proj, b_proj):
    x = np.asarray(x, np.float32)
    W_attn = np.asarray(W_attn, np.float32)
    b_attn = np.asarray(b_attn, np.float32)
    W_proj = np.asarray(W_proj, np.float32)
    b_proj = np.asarray(b_proj, np.float32)

    xT = np.ascontiguousarray(x.reshape(R, C).T)
    tri = np.triu(np.ones((KA, KA), np.float32))  # [kk, j]: 1 if j >= kk
    zeros_bp = np.zeros_like(b_proj)

    in_maps = []
    for core in range(NCORES):
        g0 = core * HL * HS  # first local column in head space
        cols = slice(g0, g0 + LC)
        w_local = np.concatenate(
            [W_attn[:, i * C:(i + 1) * C][:, cols] for i in range(3)], axis=1)
        b_local = np.concatenate(
            [b_attn[i * C:(i + 1) * C][cols] for i in range(3)])
        in_maps.append({
            "xT": xT,
            "wqkv": np.ascontiguousarray(w_local),
            "bqkv": np.ascontiguousarray(b_local),
            "wp": np.ascontiguousarray(W_proj[cols, :]),
            "bp": b_proj if core == 0 else zeros_bp,
            "trimask": tri,
        })
    return in_maps


def kernel(x, W_attn, b_attn, W_proj, b_proj):
    nc = _get_nc()
    in_maps = make_in_maps(x, W_attn, b_attn, W_proj, b_proj)
    res = run_bass_kernel_spmd(nc, in_maps, list(range(NCORES)))
    acc = res.results[0]["outT"].copy()
    for corer in res.results[1:]:
        acc += corer["outT"]
    return np.ascontiguousarray(acc.T).reshape(B, T, C)


# revision 37
# speedup vs baseline: 1.5899x; 1.0279x over previous
"""Causal self-attention (B=2, T=2048, C=1024, H=16) on 8 Trainium2 cores.

Sharding: tensor-parallel over heads (2 heads/core). Each core computes
QKV projection for its heads, causal attention, and a partial c_proj
output; partials are summed on the host (b_proj is added by core 0 only).

Per-core dataflow (everything kept "K-major" so no activation transposes
are needed on the critical path):
  xT [C, B*T]  (host pre-transposes x)
  qT/kT/vT [128, B*T] = W_local^T @ x + b      (PE, fp32r)
  S^T tile [k 128, q 512] = K @ Q^T            (PE)  -- causal tiles only
  E^T = exp(S^T/8) * causal_mask               (ACT + DVE)
  y'^T [65, q 512] += [v | 1]^T @ E^T          (PE; row 64 = softmax sums)
  y_norm^T = y'^T[0:64] * bcast(1/sums)        (PE rank-1 bcast + DVE)
  partial^T [c 128, row 512] = Wp_local^T-ish  (PE) + b_proj  -> DRAM

fp32r is used for all matmuls (full PE rate at free-dim >= 256, ~1e-4
rel err vs fp32). The BIR verifier requires fp32r operands to come from
fp32r-typed producers: DRAM inputs are declared fp32r (same bits as
fp32) so plain HWDGE DMAs satisfy it; intermediates are written as
fp32r by ACT/DVE ops.
"""

import numpy as np

import concourse.bass as bass
import concourse.tile as tile
from concourse import bacc, mybir
from concourse.bass_utils import run_bass_kernel_spmd
from concourse.masks import make_identity

F32 = mybir.dt.float32
F32R = mybir.dt.float32r

B, T, C, H = 2, 2048, 1024, 16
HS = C // H            # 64 head dim
NCORES = 8
HL = H // NCORES       # 2 local heads
LC = HL * HS           # 128 local q/k/v cols
R = B * T              # 4096 rows (b, t)
KC = C // 128          # 8 contraction chunks for projections
QT = 512               # attention q tile (free dim)
NQT = T // QT          # 4
KA = 128               # attention k chunk (partition dim)
NKA = T // KA          # 16
RT = 512               # row tile for projections
NRT = R // RT          # 8
NCC = C // 128         # 8 c_proj output chunks


def build_program():
    nc = bacc.Bacc("TRN2", target_bir_lowering=False, debug=False,
                   num_devices=NCORES)

    xT = nc.dram_tensor("xT", [C, R], F32R, kind="ExternalInput").ap()
    wqkv = nc.dram_tensor("wqkv", [C, 3 * LC], F32R, kind="ExternalInput").ap()
    bqkv = nc.dram_tensor("bqkv", [3 * LC], F32, kind="ExternalInput").ap()
    wp = nc.dram_tensor("wp", [LC, C], F32R, kind="ExternalInput").ap()
    bp = nc.dram_tensor("bp", [C], F32, kind="ExternalInput").ap()
    trimask = nc.dram_tensor("trimask", [KA, KA], F32R, kind="ExternalInput").ap()
    outT = nc.dram_tensor("outT", [C, R], F32, kind="ExternalOutput").ap()

    with tile.TileContext(nc) as tc:
        with (
            tc.tile_pool(name="consts", bufs=1) as consts,
            tc.tile_pool(name="weights", bufs=1) as weights,
            tc.tile_pool(name="qkvT", bufs=1) as qkvT_pool,
            tc.tile_pool(name="xs", bufs=3) as xs_pool,
            tc.tile_pool(name="vp", bufs=2 * NKA) as vp_pool,
            tc.tile_pool(name="et", bufs=8) as et_pool,
            tc.tile_pool(name="ysb", bufs=2) as ysb_pool,
            tc.tile_pool(name="rec", bufs=2) as rec_pool,
            tc.tile_pool(name="osb", bufs=6) as osb_pool,
            tc.tile_pool(name="dscr", bufs=4, space="DRAM") as dscr_pool,
            tc.tile_pool(name="mm512", bufs=3, space="PSUM") as mm512_pool,
            tc.tile_pool(name="ytps", bufs=2, space="PSUM") as ytps_pool,
            tc.tile_pool(name="smps", bufs=3, space="PSUM") as smps_pool,
        ):
            # ---- constants ----
            identity = consts.tile([128, 128], F32)
            make_identity(nc, identity)
            ones64_f = consts.tile([1, HS], F32)
            nc.vector.memset(ones64_f, 1.0)
            ones64 = consts.tile([1, HS], F32R)
            nc.vector.tensor_copy(ones64, ones64_f)
            ones_col = consts.tile([128, 1], F32)
            nc.vector.memset(ones_col, 1.0)
            tri_sb = consts.tile([KA, KA], F32R)
            bqkv_sb = consts.tile([128, 3], F32)
            bp_sb = consts.tile([128, NCC], F32)

            # ---- weights (fp32r-typed DRAM, plain HWDGE loads) ----
            wq_sb = weights.tile([128, KC, 3 * LC], F32R)
            wq_r = wqkv.rearrange("(kc p) n -> p kc n", p=128)
            nc.sync.dma_start(out=wq_sb[:, 0:2], in_=wq_r[:, 0:2])
            nc.sync.dma_start(
                out=bqkv_sb, in_=bqkv.rearrange("(j p) -> p j", p=128))
            wp_sb = weights.tile([LC, C], F32R)

            def load_consts():
                nc.sync.dma_start(out=tri_sb, in_=trimask)
                nc.sync.dma_start(
                    out=bp_sb, in_=bp.rearrange("(j p) -> p j", p=128))
                nc.sync.dma_start(out=wp_sb, in_=wp)

            # ---- phase 1: QKV projection (transposed outputs) ----
            qT_s = qkvT_pool.tile([LC, R], F32R, tag="qT")
            kT_s = qkvT_pool.tile([LC, R], F32R, tag="kT")
            vT_s = qkvT_pool.tile([LC, R], F32R, tag="vT")
            dst_tiles = [qT_s, kT_s, vT_s]

            def qkv_load(rt):
                x_sb = xs_pool.tile([128, KC, RT], F32R, tag="xs", name=f"x_sb_rt{rt}")
                x_r = xT[:, rt * RT:(rt + 1) * RT].rearrange(
                    "(kc p) r -> p kc r", p=128)
                if rt == 0:
                    for kc in range(0, KC, 2):
                        nc.scalar.dma_start(out=x_sb[:, kc:kc + 2],
                                            in_=x_r[:, kc:kc + 2])
                else:
                    nc.scalar.dma_start(out=x_sb[:, 0:KC // 2],
                                        in_=x_r[:, 0:KC // 2])
                    nc.scalar.dma_start(out=x_sb[:, KC // 2:],
                                        in_=x_r[:, KC // 2:])
                return x_sb

            def qkv_compute(rt, x_sb):
                if rt == 0:
                    # kc-outer for the very first tile: matmuls start as soon
                    # as the first x/w chunk lands instead of after all 8
                    pss = [mm512_pool.tile([128, RT], F32, tag="mm512",
                                           name=f"qkv_ps_rt0c{col}")
                           for col in range(3)]
                    for kc in range(KC):
                        for col in range(3):
                            nc.tensor.matmul(
                                pss[col],
                                wq_sb[:, kc, col * LC:(col + 1) * LC],
                                x_sb[:, kc, :],
                                start=(kc == 0),
                                stop=(kc == KC - 1),
                            )
                    for col in range(3):
                        nc.vector.tensor_scalar_add(
                            dst_tiles[col][:, 0:RT],
                            pss[col],
                            bqkv_sb[:, col:col + 1],
                        )
                    return
                for col in range(3):
                    ps = mm512_pool.tile([128, RT], F32, tag="mm512",
                                         name=f"qkv_ps_rt{rt}c{col}")
                    for kc in range(KC):
                        nc.tensor.matmul(
                            ps,
                            wq_sb[:, kc, col * LC:(col + 1) * LC],
                            x_sb[:, kc, :],
                            start=(kc == 0),
                            stop=(kc == KC - 1),
                        )
                    # PSUM -> SBUF with per-partition bias add, rounding to f32r
                    nc.vector.tensor_scalar_add(
                        dst_tiles[col][:, rt * RT:(rt + 1) * RT],
                        ps,
                        bqkv_sb[:, col:col + 1],
                    )

            def proj_rowtile(rt, tail):
                """c_proj partial for row tile rt (needs ynT rows complete)."""
                half = RT // 2
                for cc in range(NCC):
                    if tail and cc % 2 == 1:
                        pps = smps_pool.tile([128, RT], F32, tag="sm",
                                             name=f"pps_rt{rt}c{cc}")
                    else:
                        pps = mm512_pool.tile([128, RT], F32, tag="mm512",
                                              name=f"pps_rt{rt}c{cc}")
                    nc.tensor.matmul(
                        pps,
                        wp_sb[:, cc * 128:(cc + 1) * 128],
                        ynT_s[:, rt * RT:(rt + 1) * RT],
                        start=True,
                        stop=True,
                    )
                    o_sb = osb_pool.tile([128, RT], F32, tag="osb",
                                         name=f"o_sb_rt{rt}c{cc}")
                    nc.scalar.activation(
                        o_sb[:, 0:half], pps[:, 0:half],
                        mybir.ActivationFunctionType.Identity,
                        bias=bp_sb[:, cc:cc + 1],
                    )
                    nc.vector.tensor_scalar_add(
                        o_sb[:, half:RT], pps[:, half:RT],
                        bp_sb[:, cc:cc + 1])
                    nc.sync.dma_start(
                        out=outT[cc * 128:(cc + 1) * 128,
                                 rt * RT:(rt + 1) * RT],
                        in_=o_sb,
                    )

            # ---- phase 2: attention per (b, h), interleaved with QKV/proj ----
            ynT_s = qkvT_pool.tile([LC, R], F32R, tag="ynT")
            x0 = qkv_load(0)
            nc.sync.dma_start(out=wq_sb[:, 2:4], in_=wq_r[:, 2:4])
            nc.sync.dma_start(out=wq_sb[:, 4:KC], in_=wq_r[:, 4:KC])
            qkv_compute(0, x0)
            x1 = qkv_load(1)
            load_consts()
            qkv_compute(1, x1)
            for rt in range(2, NRT // 2):
                qkv_compute(rt, qkv_load(rt))
            for b in range(B):
                base = b * T
                if b + 1 < B:
                    for rt in range((b + 1) * NRT // 2, (b + 2) * NRT // 2):
                        qkv_compute(rt, qkv_load(rt))
                for h in range(HL):
                    hsl = slice(h * HS, (h + 1) * HS)
                    # v' tiles: [k 128, 64 v-cols | ones]
                    vps = []
                    for kc in range(NKA):
                        vp = vp_pool.tile([KA, HS + 1], F32R)
                        tp = mm512_pool.tile([KA, HS], F32, tag="mm512")
                        nc.tensor.transpose(
                            tp,
                            vT_s[hsl, base + kc * KA: base + (kc + 1) * KA]
                            .bitcast(F32),
                            identity[hsl, hsl],
                        )
                        nc.vector.tensor_copy(vp[:, 0:HS], tp)
                        nc.gpsimd.tensor_copy(vp[:, HS:HS + 1], ones_col)
                        vps.append(vp)

                    # qt-outer: only one y' accumulator live at a time
                    for qt in range(NQT):
                        yp = ytps_pool.tile([HS + 1, QT], F32, tag="yt",
                                            name=f"yt_b{b}h{h}q{qt}")
                        nka_q = (qt + 1) * (QT // KA)
                        for kc in range(nka_q):
                            diag = (kc * KA // QT == qt)
                            sps = smps_pool.tile(
                                [KA, QT], F32, tag="sm",
                                name=f"sps_b{b}h{h}q{qt}k{kc}")
                            nc.tensor.matmul(
                                sps,
                                kT_s[hsl,
                                     base + kc * KA: base + (kc + 1) * KA],
                                qT_s[hsl,
                                     base + qt * QT: base + (qt + 1) * QT],
                                start=True,
                                stop=True,
                            )
                            et = et_pool.tile([KA, QT], F32R, tag="et",
                                              name=f"et_b{b}h{h}q{qt}k{kc}")
                            # columns < off of a diagonal tile are fully
                            # masked; skip them entirely (the AV matmul
                            # accumulates only the [off, QT) span).
                            off = kc * KA - qt * QT if diag else 0
                            nc.scalar.activation(
                                et[:, off:QT], sps[:, off:QT],
                                mybir.ActivationFunctionType.Exp,
                                scale=1.0 / np.sqrt(HS).item(),
                            )
                            if diag:
                                # [off, off+128) is the triangular block
                                nc.gpsimd.tensor_mul(
                                    et[:, off:off + KA],
                                    et[:, off:off + KA],
                                    tri_sb,
                                )
                            nc.tensor.matmul(
                                yp[:, off:QT],
                                vps[kc],
                                et[:, off:QT],
                                start=(kc == 0),
                                stop=(kc == nka_q - 1),
                            )

                        # normalize: y_norm^T = y'^T[0:64] * bcast(1 / sums)
                        yts = ysb_pool.tile([HS + 1, QT], F32, tag="yts",
                                            name=f"yts_b{b}h{h}q{qt}")
                        nc.vector.tensor_copy(yts, yp)
                        rec = rec_pool.tile([1, QT], F32R, tag="rec",
                                            name=f"rec_b{b}h{h}q{qt}")
                        with nc.allow_low_precision(
                                reason="fp32r reciprocal: ~1e-4 rel err ok"):
                            nc.vector.reciprocal(rec, yts[HS:HS + 1, :])
                        if qt == NQT - 1 and h == HL - 1:
                            # end of batch: nothing else keeps PE busy, and
                            # the DRAM-bounce latency would gate the final
                            # c_proj row tile -- use a rank-1 PE broadcast
                            bcp = smps_pool.tile([HS, QT], F32, tag="sm",
                                                 name=f"bcp_b{b}h{h}q{qt}")
                            nc.tensor.matmul(bcp, ones64, rec,
                                             start=True, stop=True)
                            nc.vector.tensor_mul(
                                ynT_s[hsl,
                                      base + qt * QT: base + (qt + 1) * QT],
                                yts[0:HS, :],
                                bcp,
                            )
                        else:
                            bcs = ysb_pool.tile([HS, QT], F32R, tag="bcs",
                                                name=f"bcs_b{b}h{h}q{qt}")
                            recd = dscr_pool.tile([1, QT], F32R, tag="recd",
                                                  name=f"recd_b{b}h{h}q{qt}")
                            nc.sync.dma_start(out=recd, in_=rec)
                            rec_bcast = bass.AP(
                                tensor=recd.tensor, offset=recd.offset,
                                ap=[[0, HS]] + [list(d) for d in recd.ap[1:]])
                            nc.sync.dma_start(out=bcs, in_=rec_bcast)
                            nc.vector.tensor_mul(
                                ynT_s[hsl,
                                      base + qt * QT: base + (qt + 1) * QT],
                                yts[0:HS, :],
                                bcs,
                            )
                        # c_proj row tiles interleave into the last head's
                        # attention, one qt behind the normalize that feeds
                        # them, so the PE never waits on the bcast chain and
                        # output DMA spreads across the attention window.
                        if h == HL - 1 and qt > 0:
                            proj_rowtile(b * NRT // 2 + qt - 1,
                                         tail=(b == B - 1 and qt == NQT - 1))

                # last row tile of this batch after its attention finishes
                proj_rowtile(b * NRT // 2 + NQT - 1, tail=(b == B - 1))

    nc.compile()
    return nc


_NC = None


def _get_nc():
    global _NC
    if _NC is None:
        _NC = build_program()
    return _NC


def make_in_maps(x, W_attn, b_attn, W_proj, b_proj):
    x = np.asarray(x, np.float32)
    W_attn = np.asarray(W_attn, np.float32)
    b_attn = np.asarray(b_attn, np.float32)
    W_proj = np.asarray(W_proj, np.float32)
    b_proj = np.asarray(b_proj, np.float32)

    xT = np.ascontiguousarray(x.reshape(R, C).T)
    tri = np.triu(np.ones((KA, KA), np.float32))  # [kk, j]: 1 if j >= kk
    zeros_bp = np.zeros_like(b_proj)

    in_maps = []
    for core in range(NCORES):
        g0 = core * HL * HS  # first local column in head space
        cols = slice(g0, g0 + LC)
        w_local = np.concatenate(
            [W_attn[:, i * C:(i + 1) * C][:, cols] for i in range(3)], axis=1)
        b_local = np.concatenate(
            [b_attn[i * C:(i + 1) * C][cols] for i in range(3)])
        in_maps.append({
            "xT": xT,
            "wqkv": np.ascontiguousarray(w_local),
            "bqkv": np.ascontiguousarray(b_local),
            "wp": np.ascontiguousarray(W_proj[cols, :]),
            "bp": b_proj if core == 0 else zeros_bp,
            "trimask": tri,
        })
    return in_maps


def kernel(x, W_attn, b_attn, W_proj, b_proj):
    nc = _get_nc()
    in_maps = make_in_maps(x, W_attn, b_attn, W_proj, b_proj)
    res = run_bass_kernel_spmd(nc, in_maps, list(range(NCORES)))
    acc = res.results[0]["outT"].copy()
    for corer in res.results[1:]:
        acc += corer["outT"]
    return np.ascontiguousarray(acc.T).reshape(B, T, C)


# revision 43
# speedup vs baseline: 1.5937x; 1.0024x over previous
"""Causal self-attention (B=2, T=2048, C=1024, H=16) on 8 Trainium2 cores.

Sharding: tensor-parallel over heads (2 heads/core). Each core computes
QKV projection for its heads, causal attention, and a partial c_proj
output; partials are summed on the host (b_proj is added by core 0 only).

Per-core dataflow (everything kept "K-major" so no activation transposes
are needed on the critical path):
  xT [C, B*T]  (host pre-transposes x)
  qT/kT/vT [128, B*T] = W_local^T @ x + b      (PE, fp32r)
  S^T tile [k 128, q 512] = K @ Q^T            (PE)  -- causal tiles only
  E^T = exp(S^T/8) * causal_mask               (ACT + DVE)
  y'^T [65, q 512] += [v | 1]^T @ E^T          (PE; row 64 = softmax sums)
  y_norm^T = y'^T[0:64] * bcast(1/sums)        (PE rank-1 bcast + DVE)
  partial^T [c 128, row 512] = Wp_local^T-ish  (PE) + b_proj  -> DRAM

fp32r is used for all matmuls (full PE rate at free-dim >= 256, ~1e-4
rel err vs fp32). The BIR verifier requires fp32r operands to come from
fp32r-typed producers: DRAM inputs are declared fp32r (same bits as
fp32) so plain HWDGE DMAs satisfy it; intermediates are written as
fp32r by ACT/DVE ops.
"""

import numpy as np

import concourse.bass as bass
import concourse.tile as tile
from concourse import bacc, mybir
from concourse.bass_utils import run_bass_kernel_spmd
from concourse.masks import make_identity

F32 = mybir.dt.float32
F32R = mybir.dt.float32r

B, T, C, H = 2, 2048, 1024, 16
HS = C // H            # 64 head dim
NCORES = 8
HL = H // NCORES       # 2 local heads
LC = HL * HS           # 128 local q/k/v cols
R = B * T              # 4096 rows (b, t)
KC = C // 128          # 8 contraction chunks for projections
QT = 512               # attention q tile (free dim)
NQT = T // QT          # 4
KA = 128               # attention k chunk (partition dim)
NKA = T // KA          # 16
RT = 512               # row tile for projections
NRT = R // RT          # 8
NCC = C // 128         # 8 c_proj output chunks


def build_program():
    nc = bacc.Bacc("TRN2", target_bir_lowering=False, debug=False,
                   num_devices=NCORES)

    xT = nc.dram_tensor("xT", [C, R], F32R, kind="ExternalInput").ap()
    wqkv = nc.dram_tensor("wqkv", [C, 3 * LC], F32R, kind="ExternalInput").ap()
    bqkv = nc.dram_tensor("bqkv", [3 * LC], F32, kind="ExternalInput").ap()
    wp = nc.dram_tensor("wp", [LC, C], F32R, kind="ExternalInput").ap()
    bp = nc.dram_tensor("bp", [C], F32, kind="ExternalInput").ap()
    trimask = nc.dram_tensor("trimask", [KA, KA], F32R, kind="ExternalInput").ap()
    outT = nc.dram_tensor("outT", [C, R], F32, kind="ExternalOutput").ap()

    with tile.TileContext(nc) as tc:
        with (
            tc.tile_pool(name="consts", bufs=1) as consts,
            tc.tile_pool(name="weights", bufs=1) as weights,
            tc.tile_pool(name="qkvT", bufs=1) as qkvT_pool,
            tc.tile_pool(name="xs", bufs=3) as xs_pool,
            tc.tile_pool(name="vp", bufs=2 * NKA) as vp_pool,
            tc.tile_pool(name="et", bufs=8) as et_pool,
            tc.tile_pool(name="ysb", bufs=2) as ysb_pool,
            tc.tile_pool(name="rec", bufs=2) as rec_pool,
            tc.tile_pool(name="osb", bufs=6) as osb_pool,
            tc.tile_pool(name="dscr", bufs=4, space="DRAM") as dscr_pool,
            tc.tile_pool(name="mm512", bufs=3, space="PSUM") as mm512_pool,
            tc.tile_pool(name="ytps", bufs=2, space="PSUM") as ytps_pool,
            tc.tile_pool(name="smps", bufs=3, space="PSUM") as smps_pool,
        ):
            # ---- constants ----
            identity = consts.tile([128, 128], F32)
            make_identity(nc, identity)
            ones64_f = consts.tile([1, HS], F32)
            nc.vector.memset(ones64_f, 1.0)
            ones64 = consts.tile([1, HS], F32R)
            nc.vector.tensor_copy(ones64, ones64_f)
            ones_col = consts.tile([128, 1], F32)
            nc.vector.memset(ones_col, 1.0)
            tri_sb = consts.tile([KA, KA], F32R)
            bqkv_sb = consts.tile([128, 3], F32)
            bp_sb = consts.tile([128, NCC], F32)

            # ---- weights (fp32r-typed DRAM, plain HWDGE loads) ----
            wq_sb = weights.tile([128, KC, 3 * LC], F32R)
            wq_r = wqkv.rearrange("(kc p) n -> p kc n", p=128)
            nc.sync.dma_start(out=wq_sb[:, 0:2], in_=wq_r[:, 0:2])
            nc.sync.dma_start(
                out=bqkv_sb, in_=bqkv.rearrange("(j p) -> p j", p=128))
            wp_sb = weights.tile([LC, C], F32R)

            def load_consts():
                nc.sync.dma_start(out=tri_sb, in_=trimask)
                nc.sync.dma_start(
                    out=bp_sb, in_=bp.rearrange("(j p) -> p j", p=128))
                nc.sync.dma_start(out=wp_sb, in_=wp)

            # ---- phase 1: QKV projection (transposed outputs) ----
            qT_s = qkvT_pool.tile([LC, R], F32R, tag="qT")
            kT_s = qkvT_pool.tile([LC, R], F32R, tag="kT")
            vT_s = qkvT_pool.tile([LC, R], F32R, tag="vT")
            dst_tiles = [qT_s, kT_s, vT_s]

            def qkv_load(rt):
                x_sb = xs_pool.tile([128, KC, RT], F32R, tag="xs", name=f"x_sb_rt{rt}")
                x_r = xT[:, rt * RT:(rt + 1) * RT].rearrange(
                    "(kc p) r -> p kc r", p=128)
                if rt == 0:
                    for kc in range(0, KC, 2):
                        nc.scalar.dma_start(out=x_sb[:, kc:kc + 2],
                                            in_=x_r[:, kc:kc + 2])
                else:
                    nc.scalar.dma_start(out=x_sb[:, 0:KC // 2],
                                        in_=x_r[:, 0:KC // 2])
                    nc.scalar.dma_start(out=x_sb[:, KC // 2:],
                                        in_=x_r[:, KC // 2:])
                return x_sb

            def qkv_compute(rt, x_sb):
                if rt == 0:
                    # kc-outer for the very first tile: matmuls start as soon
                    # as the first x/w chunk lands instead of after all 8
                    pss = [mm512_pool.tile([128, RT], F32, tag="mm512",
                                           name=f"qkv_ps_rt0c{col}")
                           for col in range(3)]
                    for kc in range(KC):
                        for col in range(3):
                            nc.tensor.matmul(
                                pss[col],
                                wq_sb[:, kc, col * LC:(col + 1) * LC],
                                x_sb[:, kc, :],
                                start=(kc == 0),
                                stop=(kc == KC - 1),
                            )
                    for col in range(3):
                        nc.vector.tensor_scalar_add(
                            dst_tiles[col][:, 0:RT],
                            pss[col],
                            bqkv_sb[:, col:col + 1],
                        )
                    return
                for col in range(3):
                    ps = mm512_pool.tile([128, RT], F32, tag="mm512",
                                         name=f"qkv_ps_rt{rt}c{col}")
                    for kc in range(KC):
                        nc.tensor.matmul(
                            ps,
                            wq_sb[:, kc, col * LC:(col + 1) * LC],
                            x_sb[:, kc, :],
                            start=(kc == 0),
                            stop=(kc == KC - 1),
                        )
                    # PSUM -> SBUF with per-partition bias add, rounding to f32r
                    nc.vector.tensor_scalar_add(
                        dst_tiles[col][:, rt * RT:(rt + 1) * RT],
                        ps,
                        bqkv_sb[:, col:col + 1],
                    )

            def proj_rowtile(rt, tail):
                """c_proj partial for row tile rt (needs ynT rows complete)."""
                half = RT // 2
                for cc in range(NCC):
                    if tail and cc % 2 == 1:
                        pps = smps_pool.tile([128, RT], F32, tag="sm",
                                             name=f"pps_rt{rt}c{cc}")
                    else:
                        pps = mm512_pool.tile([128, RT], F32, tag="mm512",
                                              name=f"pps_rt{rt}c{cc}")
                    nc.tensor.matmul(
                        pps,
                        wp_sb[:, cc * 128:(cc + 1) * 128],
                        ynT_s[:, rt * RT:(rt + 1) * RT],
                        start=True,
                        stop=True,
                    )
                    o_sb = osb_pool.tile([128, RT], F32, tag="osb",
                                         name=f"o_sb_rt{rt}c{cc}")
                    nc.scalar.activation(
                        o_sb[:, 0:half], pps[:, 0:half],
                        mybir.ActivationFunctionType.Identity,
                        bias=bp_sb[:, cc:cc + 1],
                    )
                    nc.vector.tensor_scalar_add(
                        o_sb[:, half:RT], pps[:, half:RT],
                        bp_sb[:, cc:cc + 1])
                    nc.sync.dma_start(
                        out=outT[cc * 128:(cc + 1) * 128,
                                 rt * RT:(rt + 1) * RT],
                        in_=o_sb,
                    )

            # ---- phase 2: attention per (b, h), interleaved with QKV/proj ----
            ynT_s = qkvT_pool.tile([LC, R], F32R, tag="ynT")
            x0 = qkv_load(0)
            nc.sync.dma_start(out=wq_sb[:, 2:4], in_=wq_r[:, 2:4])
            nc.sync.dma_start(out=wq_sb[:, 4:KC], in_=wq_r[:, 4:KC])
            qkv_compute(0, x0)
            x1 = qkv_load(1)
            load_consts()
            qkv_compute(1, x1)
            for rt in range(2, NRT // 2):
                qkv_compute(rt, qkv_load(rt))
            for b in range(B):
                base = b * T
                if b + 1 < B:
                    for rt in range((b + 1) * NRT // 2, (b + 2) * NRT // 2):
                        qkv_compute(rt, qkv_load(rt))
                for h in range(HL):
                    hsl = slice(h * HS, (h + 1) * HS)
                    vps = []

                    # qt-outer: only one y' accumulator live at a time
                    for qt in range(NQT):
                        # v' tiles [k 128, 64 v-cols | ones] for the k chunks
                        # this qt introduces -- lazy prep keeps (h, qt)
                        # dependent only on QKV row tiles <= qt
                        for kc in range(qt * (QT // KA),
                                        (qt + 1) * (QT // KA)):
                            vp = vp_pool.tile([KA, HS + 1], F32R,
                                              name=f"vp_b{b}h{h}k{kc}",
                                              tag="vp")
                            tp = mm512_pool.tile([KA, HS], F32, tag="mm512",
                                                 name=f"tp_b{b}h{h}k{kc}")
                            nc.tensor.transpose(
                                tp,
                                vT_s[hsl,
                                     base + kc * KA: base + (kc + 1) * KA]
                                .bitcast(F32),
                                identity[hsl, hsl],
                            )
                            nc.vector.tensor_copy(vp[:, 0:HS], tp)
                            nc.gpsimd.tensor_copy(vp[:, HS:HS + 1], ones_col)
                            vps.append(vp)
                        yp = ytps_pool.tile([HS + 1, QT], F32, tag="yt",
                                            name=f"yt_b{b}h{h}q{qt}")
                        nka_q = (qt + 1) * (QT // KA)
                        for kc in range(nka_q):
                            diag = (kc * KA // QT == qt)
                            sps = smps_pool.tile(
                                [KA, QT], F32, tag="sm",
                                name=f"sps_b{b}h{h}q{qt}k{kc}")
                            nc.tensor.matmul(
                                sps,
                                kT_s[hsl,
                                     base + kc * KA: base + (kc + 1) * KA],
                                qT_s[hsl,
                                     base + qt * QT: base + (qt + 1) * QT],
                                start=True,
                                stop=True,
                            )
                            et = et_pool.tile([KA, QT], F32R, tag="et",
                                              name=f"et_b{b}h{h}q{qt}k{kc}")
                            # columns < off of a diagonal tile are fully
                            # masked; skip them entirely (the AV matmul
                            # accumulates only the [off, QT) span).
                            off = kc * KA - qt * QT if diag else 0
                            nc.scalar.activation(
                                et[:, off:QT], sps[:, off:QT],
                                mybir.ActivationFunctionType.Exp,
                                scale=1.0 / np.sqrt(HS).item(),
                            )
                            if diag:
                                # [off, off+128) is the triangular block
                                nc.gpsimd.tensor_mul(
                                    et[:, off:off + KA],
                                    et[:, off:off + KA],
                                    tri_sb,
                                )
                            nc.tensor.matmul(
                                yp[:, off:QT],
                                vps[kc],
                                et[:, off:QT],
                                start=(kc == 0),
                                stop=(kc == nka_q - 1),
                            )

                        # normalize: y_norm^T = y'^T[0:64] * bcast(1 / sums)
                        yts = ysb_pool.tile([HS + 1, QT], F32, tag="yts",
                                            name=f"yts_b{b}h{h}q{qt}")
                        nc.vector.tensor_copy(yts, yp)
                        rec = rec_pool.tile([1, QT], F32R, tag="rec",
                                            name=f"rec_b{b}h{h}q{qt}")
                        with nc.allow_low_precision(
                                reason="fp32r reciprocal: ~1e-4 rel err ok"):
                            nc.vector.reciprocal(rec, yts[HS:HS + 1, :])
                        if qt == NQT - 1 and h == HL - 1:
                            # end of batch: nothing else keeps PE busy, and
                            # the DRAM-bounce latency would gate the final
                            # c_proj row tile -- use a rank-1 PE broadcast
                            bcp = smps_pool.tile([HS, QT], F32, tag="sm",
                                                 name=f"bcp_b{b}h{h}q{qt}")
                            nc.tensor.matmul(bcp, ones64, rec,
                                             start=True, stop=True)
                            nc.vector.tensor_mul(
                                ynT_s[hsl,
                                      base + qt * QT: base + (qt + 1) * QT],
                                yts[0:HS, :],
                                bcp,
                            )
                        else:
                            bcs = ysb_pool.tile([HS, QT], F32R, tag="bcs",
                                                name=f"bcs_b{b}h{h}q{qt}")
                            recd = dscr_pool.tile([1, QT], F32R, tag="recd",
                                                  name=f"recd_b{b}h{h}q{qt}")
                            nc.sync.dma_start(out=recd, in_=rec)
                            rec_bcast = bass.AP(
                                tensor=recd.tensor, offset=recd.offset,
                                ap=[[0, HS]] + [list(d) for d in recd.ap[1:]])
                            nc.sync.dma_start(out=bcs, in_=rec_bcast)
                            nc.vector.tensor_mul(
                                ynT_s[hsl,
                                      base + qt * QT: base + (qt + 1) * QT],
                                yts[0:HS, :],
                                bcs,
                            )
                        # c_proj row tiles interleave into the last head's
                        # attention, one qt behind the normalize that feeds
                        # them, so the PE never waits on the bcast chain and
                        # output DMA spreads across the attention window.
                        if h == HL - 1 and qt > 0:
                            proj_rowtile(b * NRT // 2 + qt - 1,
                                         tail=(b == B - 1 and qt == NQT - 1))

                # last row tile of this batch after its attention finishes
                proj_rowtile(b * NRT // 2 + NQT - 1, tail=(b == B - 1))

    nc.compile()
    return nc


_NC = None


def _get_nc():
    global _NC
    if _NC is None:
        _NC = build_program()
    return _NC


def make_in_maps(x, W_attn, b_attn, W_proj, b_proj):
    x = np.asarray(x, np.float32)
    W_attn = np.asarray(W_attn, np.float32)
    b_attn = np.asarray(b_attn, np.float32)
    W_proj = np.asarray(W_proj, np.float32)
    b_proj = np.asarray(b_proj, np.float32)

    xT = np.ascontiguousarray(x.reshape(R, C).T)
    tri = np.triu(np.ones((KA, KA), np.float32))  # [kk, j]: 1 if j >= kk
    zeros_bp = np.zeros_like(b_proj)

    in_maps = []
    for core in range(NCORES):
        g0 = core * HL * HS  # first local column in head space
        cols = slice(g0, g0 + LC)
        w_local = np.concatenate(
            [W_attn[:, i * C:(i + 1) * C][:, cols] for i in range(3)], axis=1)
        b_local = np.concatenate(
            [b_attn[i * C:(i + 1) * C][cols] for i in range(3)])
        in_maps.append({
            "xT": xT,
            "wqkv": np.ascontiguousarray(w_local),
            "bqkv": np.ascontiguousarray(b_local),
            "wp": np.ascontiguousarray(W_proj[cols, :]),
            "bp": b_proj if core == 0 else zeros_bp,
            "trimask": tri,
        })
    return in_maps


def kernel(x, W_attn, b_attn, W_proj, b_proj):
    nc = _get_nc()
    in_maps = make_in_maps(x, W_attn, b_attn, W_proj, b_proj)
    res = run_bass_kernel_spmd(nc, in_maps, list(range(NCORES)))
    acc = res.results[0]["outT"].copy()
    for corer in res.results[1:]:
        acc += corer["outT"]
    return np.ascontiguousarray(acc.T).reshape(B, T, C)
